# revision 7
# baseline (speedup 1.0000x reference)
"""Causal self-attention (B=4,T=2048,C=2048,H=16,D=128) on 8 TRN2 NeuronCores.

Strategy: tensor-parallel over heads (2 heads/core) for QKV + attention,
AllToAll to redistribute y^T so each core holds all channels for 1/8 of the
(b,t) rows, then t-sharded output projection. bf16 matmuls, fp32 PSUM
accumulation, softmax without max-subtraction (scores ~ N(0,1)), causal
masking via precomputed 0/1 tiles multiplied post-exp, RoPE via host-side
weight-row permutation + 6 DVE ops per tensor.
"""
import os
import sys

sys.path.insert(0, "/opt/trn_rl_repo")

import numpy as np
import ml_dtypes

B, T, C, H, D = 4, 2048, 2048, 16, 128
NCORES = 8
HPC = H // NCORES          # 2 heads per core
BT = B * T                 # 8192
BTPC = BT // NCORES        # 1024 (b,t) rows per core for out-proj
KT = C // 128              # 16 contraction tiles
SCALE = 1.0 / float(np.sqrt(D))
BF16 = ml_dtypes.bfloat16

LAST_EXEC_NS = None
_CACHE = {}


def _build_nc():
    from contextlib import ExitStack
    from concourse import bacc, tile, mybir
    import concourse.bass as bass  # noqa: F401

    bf = mybir.dt.bfloat16
    f32 = mybir.dt.float32
    mult = mybir.AluOpType.mult
    add = mybir.AluOpType.add
    sub = mybir.AluOpType.subtract
    Exp = mybir.ActivationFunctionType.Exp

    nc = bacc.Bacc("TRN2", target_bir_lowering=False, debug=False,
                   num_devices=NCORES)

    xT_d = nc.dram_tensor("xT", [C, BT], bf, kind="ExternalInput")
    wq_d = nc.dram_tensor("wqT", [C, HPC * D], bf, kind="ExternalInput")
    wk_d = nc.dram_tensor("wkT", [C, HPC * D], bf, kind="ExternalInput")
    wv_d = nc.dram_tensor("wvT", [C, HPC * D], bf, kind="ExternalInput")
    # CC = [cos; cos], SS = [-sin; sin] stacked to 128 partitions so every
    # RoPE op uses identical partition ranges (BIR samePartitionsAll).
    cos_d = nc.dram_tensor("ccT", [D, T], bf, kind="ExternalInput")
    sin_d = nc.dram_tensor("ssT", [D, T], bf, kind="ExternalInput")
    mask_d = nc.dram_tensor("masks", [128, 2048], bf, kind="ExternalInput")
    wp_d = nc.dram_tensor("wpT", [C, C], bf, kind="ExternalInput")
    out_d = nc.dram_tensor("outT", [C, BTPC], f32, kind="ExternalOutput")

    with tile.TileContext(nc) as tc:
        with tc.tile_pool(name="dram", bufs=1, space="DRAM") as dram:
            a2a_in = dram.tile([2048, BTPC], bf)
            a2a_out = dram.tile([2048, BTPC], bf)

            with ExitStack() as ab:
                const = ab.enter_context(tc.tile_pool(name="const", bufs=1))
                wpool = ab.enter_context(tc.tile_pool(name="w", bufs=KT))
                xt_pool = ab.enter_context(tc.tile_pool(name="xt", bufs=32))
                qkraw_pool = ab.enter_context(tc.tile_pool(name="qkraw", bufs=5))
                rtmp_pool = ab.enter_context(tc.tile_pool(name="rtmp", bufs=2))
                rot_pool = ab.enter_context(tc.tile_pool(name="rot", bufs=3))
                v_pool = ab.enter_context(tc.tile_pool(name="v", bufs=32))
                exp_pool = ab.enter_context(tc.tile_pool(name="expp", bufs=2))
                expm_pool = ab.enter_context(tc.tile_pool(name="expm", bufs=2))
                acc_pool = ab.enter_context(tc.tile_pool(name="acc", bufs=2))
                norm_pool = ab.enter_context(tc.tile_pool(name="norm", bufs=2))
                ps_pool = ab.enter_context(
                    tc.tile_pool(name="ps", bufs=1, space="PSUM"))

                # constants
                cos_sb = const.tile([D, T], bf, name="cos_sb")
                nc.sync.dma_start(cos_sb[:], cos_d.ap())
                sin_sb = const.tile([D, T], bf, name="sin_sb")
                nc.sync.dma_start(sin_sb[:], sin_d.ap())
                mask_sb = const.tile([128, 2048], bf, name="mask_sb")
                nc.sync.dma_start(mask_sb[:], mask_d.ap())
                ones_sb = const.tile([128, 128], bf, name="ones_sb")
                nc.vector.memset(ones_sb[:], 1.0)

                wq_sb, wk_sb, wv_sb = [], [], []
                for kk in range(KT):
                    for (lst, d_, tag) in ((wq_sb, wq_d, "wq"),
                                           (wk_sb, wk_d, "wk"),
                                           (wv_sb, wv_d, "wv")):
                        w_ = wpool.tile([128, HPC * D], bf,
                                        name=f"{tag}_{kk}", tag=tag)
                        nc.sync.dma_start(
                            w_[:], d_.ap()[128 * kk:128 * kk + 128, :])
                        lst.append(w_)

                for b in range(B):
                    # ---- load x^T tiles for this batch ----
                    xt = {}
                    for tt4 in range(4):
                        for kk in range(KT):
                            t_ = xt_pool.tile(
                                [128, 512], bf,
                                name=f"xt_{b}_{tt4}_{kk}", tag="xt")
                            c0 = 2048 * b + 512 * tt4
                            nc.sync.dma_start(
                                t_[:],
                                xT_d.ap()[128 * kk:128 * kk + 128, c0:c0 + 512])
                            xt[(kk, tt4)] = t_

                    qraw, kraw = {}, {}
                    for l in range(HPC):
                        qraw[l] = qkraw_pool.tile(
                            [128, T], bf, name=f"qraw_{b}_{l}", tag="qkraw")
                        kraw[l] = qkraw_pool.tile(
                            [128, T], bf, name=f"kraw_{b}_{l}", tag="qkraw")

                    vt = {}
                    for tt4 in range(4):
                        # v tiles (t-major), both heads at once
                        for s in range(4):
                            tt = 4 * tt4 + s
                            vps = ps_pool.tile([128, HPC * D], f32,
                                               name=f"vps_{b}_{tt}",
                                               tag="psqkv", bufs=2)
                            for kk in range(KT):
                                nc.tensor.matmul(
                                    vps[:],
                                    xt[(kk, tt4)][:, 128 * s:128 * s + 128],
                                    wv_sb[kk][:],
                                    start=(kk == 0), stop=(kk == KT - 1))
                            v_ = v_pool.tile([128, HPC * D], bf,
                                             name=f"v_{b}_{tt}", tag="v")
                            nc.scalar.copy(v_[:], vps[:])
                            vt[tt] = v_
                        # q^T / k^T (d-major)
                        for l in range(HPC):
                            for (wsb, raw, nm) in ((wq_sb, qraw[l], "q"),
                                                   (wk_sb, kraw[l], "k")):
                                ps = ps_pool.tile(
                                    [128, 512], f32,
                                    name=f"{nm}ps_{b}_{l}_{tt4}",
                                    tag="psqkv", bufs=2)
                                for kk in range(KT):
                                    nc.tensor.matmul(
                                        ps[:],
                                        wsb[kk][:, 128 * l:128 * l + 128],
                                        xt[(kk, tt4)][:],
                                        start=(kk == 0), stop=(kk == KT - 1))
                                nc.scalar.copy(
                                    raw[:, 512 * tt4:512 * tt4 + 512], ps[:])

                    # ---- RoPE ----
                    # raw = [q1; q2] (perm-major halves). rot = raw*[cos;cos]
                    # + swap(raw)*[-sin;sin], swap via 2 SBUF-SBUF DMAs so all
                    # DVE ops keep identical partition ranges.
                    rot = {}
                    for l in range(HPC):
                        for (raw, tag) in ((qraw[l], "qrot"), (kraw[l], "krot")):
                            sw = rtmp_pool.tile([128, T], bf,
                                                name=f"sw_{tag}_{b}_{l}",
                                                tag="sw")
                            nc.sync.dma_start(sw[0:64, :], raw[64:128, :])
                            nc.sync.dma_start(sw[64:128, :], raw[0:64, :])
                            r_ = rot_pool.tile([128, T], bf,
                                               name=f"{tag}_{b}_{l}", tag=tag)
                            w_ = rtmp_pool.tile([128, T], bf,
                                                name=f"w_{tag}_{b}_{l}",
                                                tag="wt")
                            nc.vector.tensor_tensor(
                                r_[:], raw[:], cos_sb[:], op=mult)
                            nc.vector.tensor_tensor(
                                w_[:], sw[:], sin_sb[:], op=mult)
                            nc.vector.tensor_tensor(
                                r_[:], r_[:], w_[:], op=add)
                            rot[(tag, l)] = r_

                    # ---- attention per local head ----
                    for l in range(HPC):
                        qrot = rot[("qrot", l)]
                        krot = rot[("krot", l)]
                        for jj in range(4):      # tq tile of 512
                            yps = ps_pool.tile([128, 512], f32,
                                               name=f"yps_{b}_{l}_{jj}",
                                               tag="psy", bufs=1)
                            acc = acc_pool.tile([128, 512], bf,
                                                name=f"acc_{b}_{l}_{jj}",
                                                tag="acc")
                            for g in range(jj + 1):   # tk groups of 4x128
                                scps = ps_pool.tile(
                                    [128, 2048], f32,
                                    name=f"sc_{b}_{l}_{jj}_{g}",
                                    tag="pssc", bufs=1)
                                for i in range(4):
                                    tk = 4 * g + i
                                    nc.tensor.matmul(
                                        scps[:, 512 * i:512 * i + 512],
                                        krot[:, 128 * tk:128 * tk + 128],
                                        qrot[:, 512 * jj:512 * jj + 512],
                                        start=True, stop=True)
                                ex = exp_pool.tile(
                                    [128, 2048], bf,
                                    name=f"ex_{b}_{l}_{jj}_{g}", tag="ex")
                                nc.scalar.activation(
                                    ex[:], scps[:], Exp, scale=SCALE)
                                if g == jj:
                                    exm = expm_pool.tile(
                                        [128, 2048], bf,
                                        name=f"exm_{b}_{l}_{jj}", tag="exm")
                                    nc.vector.tensor_tensor(
                                        exm[:], ex[:], mask_sb[:], op=mult)
                                    ex = exm
                                for i in range(4):
                                    if g == 0 and i == 0:
                                        nc.vector.tensor_copy(
                                            acc[:], ex[:, 0:512])
                                    else:
                                        nc.vector.tensor_tensor(
                                            acc[:], acc[:],
                                            ex[:, 512 * i:512 * i + 512],
                                            op=add)
                                for i in range(4):
                                    tk = 4 * g + i
                                    nc.tensor.matmul(
                                        yps[:],
                                        vt[tk][:, 128 * l:128 * l + 128],
                                        ex[:, 512 * i:512 * i + 512],
                                        start=(g == 0 and i == 0),
                                        stop=(g == jj and i == 3))
                            sums = ps_pool.tile([128, 512], f32,
                                                name=f"sums_{b}_{l}_{jj}",
                                                tag="pssum", bufs=1)
                            nc.tensor.matmul(sums[:], ones_sb[:], acc[:],
                                             start=True, stop=True)
                            rec = norm_pool.tile([128, 512], f32,
                                                 name=f"rec_{b}_{l}_{jj}",
                                                 tag="rec")
                            nc.vector.reciprocal(rec[:], sums[:])
                            yn = norm_pool.tile([128, 512], bf,
                                                name=f"yn_{b}_{l}_{jj}",
                                                tag="yn")
                            nc.vector.tensor_tensor(
                                yn[:], yps[:], rec[:], op=mult)
                            chunk = 2 * b + jj // 2
                            r0 = 256 * chunk + 128 * l
                            c0 = 512 * (jj % 2)
                            nc.sync.dma_start(
                                a2a_in[r0:r0 + 128, c0:c0 + 512], yn[:])

            # ---- AllToAll: out block j = core j's in block (this core) ----
            nc.gpsimd.collective_compute(
                "AllToAll",
                mybir.AluOpType.bypass,
                replica_groups=[list(range(NCORES))],
                ins=[a2a_in.opt()],
                outs=[a2a_out.opt()],
            )

            # ---- output projection (t-sharded) ----
            with ExitStack() as pc:
                wp_pool = pc.enter_context(tc.tile_pool(name="wp", bufs=KT))
                y2_pool = pc.enter_context(tc.tile_pool(name="y2", bufs=KT))
                ob_pool = pc.enter_context(tc.tile_pool(name="ob", bufs=4))
                ps_op = pc.enter_context(
                    tc.tile_pool(name="psop", bufs=4, space="PSUM"))

                wp_sb, y2_sb = [], []
                for kk in range(KT):
                    w_ = wp_pool.tile([128, C], bf, name=f"wp_{kk}", tag="wp")
                    nc.sync.dma_start(
                        w_[:], wp_d.ap()[128 * kk:128 * kk + 128, :])
                    wp_sb.append(w_)
                    y_ = y2_pool.tile([128, BTPC], bf,
                                      name=f"y2_{kk}", tag="y2")
                    nc.sync.dma_start(
                        y_[:], a2a_out[128 * kk:128 * kk + 128, :])
                    y2_sb.append(y_)
                for ff in range(KT):
                    for tt in range(BTPC // 512):
                        ps = ps_op.tile([128, 512], f32,
                                        name=f"ops_{ff}_{tt}", tag="psop")
                        for kk in range(KT):
                            nc.tensor.matmul(
                                ps[:],
                                wp_sb[kk][:, 128 * ff:128 * ff + 128],
                                y2_sb[kk][:, 512 * tt:512 * tt + 512],
                                start=(kk == 0), stop=(kk == KT - 1))
                        ob = ob_pool.tile([128, 512], f32,
                                          name=f"ob_{ff}_{tt}", tag="ob")
                        nc.scalar.copy(ob[:], ps[:])
                        nc.sync.dma_start(
                            out_d.ap()[128 * ff:128 * ff + 128,
                                       512 * tt:512 * tt + 512], ob[:])

    nc.compile()
    return nc


def _prep_inputs(x, rope_freqs, W_attn, W_proj):
    x = np.asarray(x, np.float32)
    rope_freqs = np.asarray(rope_freqs, np.float32)
    W_attn = np.asarray(W_attn, np.float32)
    W_proj = np.asarray(W_proj, np.float32)

    xT = np.ascontiguousarray(x.reshape(BT, C).T).astype(BF16)
    perm = np.concatenate([np.arange(0, D, 2), np.arange(1, D, 2)])
    theta = np.outer(rope_freqs.astype(np.float64), np.arange(T))
    cos_, sin_ = np.cos(theta), np.sin(theta)
    ccT = np.concatenate([cos_, cos_], axis=0).astype(BF16)   # (128, T)
    ssT = np.concatenate([-sin_, sin_], axis=0).astype(BF16)  # (128, T)
    masks = np.zeros((128, 2048), np.float32)
    for i in range(4):
        masks[:, 512 * i:512 * (i + 1)] = (
            np.arange(512)[None, :] >= (np.arange(128)[:, None] + 128 * i))
    masks = masks.astype(BF16)
    wpT = np.ascontiguousarray(W_proj.T).astype(BF16)

    in_maps = []
    for r in range(NCORES):
        wq_rows, wk_rows, wv_rows = [], [], []
        for l in range(HPC):
            h = HPC * r + l
            wq_rows.append(W_attn[D * h:D * h + D][perm])
            wk_rows.append(W_attn[C + D * h:C + D * h + D][perm])
            wv_rows.append(W_attn[2 * C + D * h:2 * C + D * h + D])
        in_maps.append({
            "xT": xT,
            "wqT": np.ascontiguousarray(
                np.concatenate(wq_rows, 0).T).astype(BF16),
            "wkT": np.ascontiguousarray(
                np.concatenate(wk_rows, 0).T).astype(BF16),
            "wvT": np.ascontiguousarray(
                np.concatenate(wv_rows, 0).T).astype(BF16),
            "ccT": ccT,
            "ssT": ssT,
            "masks": masks,
            "wpT": wpT,
        })
    return in_maps


def _ensure_trace_support():
    """Register the axon NTFF profiling hook if the image's antenv lacks it,
    and stub out the artifact upload (no bucket access in-container)."""
    import types
    import sys as _sys
    import antenv

    if "antenv.axon_hooks" not in _sys.modules:
        try:
            import antenv.axon_hooks  # noqa: F401
        except ImportError:
            mod = types.ModuleType("antenv.axon_hooks")
            _holder = {}
            mod.set_axon_ntff_profile_hook = (
                lambda h: _holder.__setitem__("h", h))
            mod.get_axon_ntff_profile_hook = lambda: _holder.get("h")
            _sys.modules["antenv.axon_hooks"] = mod
            antenv.axon_hooks = mod
    import antenv.axon_hooks as ah

    if ah.get_axon_ntff_profile_hook() is None:
        try:
            from trn_agent_boot.trn_boot import _ntff_profile_via_ctypes
            hook = _ntff_profile_via_ctypes("/opt/axon/libaxon_pjrt.so")
            if hook is not None:
                ah.set_axon_ntff_profile_hook(hook)
        except Exception as e:  # profiling stays off; run still works
            print(f"ntff hook registration failed: {e}", file=sys.stderr)
    from concourse import bass_utils as bu
    bu.upload_artifacts = lambda tmpdir: f"local://{tmpdir}"


def kernel(x, rope_freqs, W_attn, W_proj):
    global LAST_EXEC_NS
    from concourse import bass_utils

    if "nc" not in _CACHE:
        _CACHE["nc"] = _build_nc()
    nc = _CACHE["nc"]

    in_maps = _prep_inputs(x, rope_freqs, W_attn, W_proj)
    trace = os.environ.get("KERNEL_TRACE", "0") == "1"
    tmpdir = None
    if trace:
        _ensure_trace_support()
        tmpdir = os.environ.get("KERNEL_TRACE_DIR") or None
    res = bass_utils.run_bass_kernel_spmd(
        nc, in_maps, core_ids=list(range(NCORES)), trace=trace,
        tmpdir=tmpdir)
    LAST_EXEC_NS = res.exec_time_ns

    outT = np.concatenate(
        [np.asarray(res.results[r]["outT"], np.float32)
         for r in range(NCORES)], axis=1)
    return np.ascontiguousarray(outT.T).reshape(B, T, C)
